# revision 37
# baseline (speedup 1.0000x reference)
"""Trainium2 Bass kernel for the gnn_message_passing problem (nn_Att_87411174408394).

Strategy: shard edges by destination-node (hi) range across 8 cores; each core
owns a contiguous 12500-node shard of `agts`, so the index_add scatter is fully
core-local (no collectives). ctx table is replicated (bf16) and gathered with
batched indirect DMA. Edge MLPs run in bf16 on the TensorEngine; GroupNorms use
bn_stats + fused scale/bias activations. The scatter is a one-hot matmul into
PSUM per 128-node block (edges are sorted by hi on the host, so each 128-edge
chunk hits exactly one node block).
"""

import math
import sys

import numpy as np

sys.path.insert(0, "/opt/trn_rl_repo")

import ml_dtypes  # noqa: E402
import concourse.bass as bass  # noqa: E402
import concourse.tile as tile  # noqa: E402
from concourse import library_config, mybir  # noqa: E402
from concourse.bass_utils import run_bass_kernel_spmd  # noqa: E402

BF16 = mybir.dt.bfloat16
F32 = mybir.dt.float32
I32 = mybir.dt.int32
I16 = mybir.dt.int16
NPBF16 = ml_dtypes.bfloat16

P = 128
EPS = 1e-5
N_CORES = 8


def _install_ntff_hook_shim():
    """The agent image's antenv lacks axon_hooks; recreate it from the boot
    helpers so run_bass_kernel_spmd(trace=True) can capture NTFF profiles."""
    try:
        import antenv  # noqa: PLC0415

        try:
            import antenv.axon_hooks  # noqa: F401, PLC0415

            return
        except ImportError:
            pass
        import types  # noqa: PLC0415

        from trn_agent_boot.trn_boot import _ntff_profile_via_ctypes  # noqa: PLC0415

        hook = _ntff_profile_via_ctypes("/opt/axon/libaxon_pjrt.so")
        mod = types.ModuleType("antenv.axon_hooks")
        mod._hook = hook
        mod.get_axon_ntff_profile_hook = lambda: mod._hook
        mod.set_axon_ntff_profile_hook = lambda h: setattr(mod, "_hook", h)
        sys.modules["antenv.axon_hooks"] = mod
        antenv.axon_hooks = mod
    except Exception:
        pass


_install_ntff_hook_shim()


def _patch_bir_sem_clear(bir: bytes) -> bytes:
    """This image's walrus rejects the EVENT_SEMAPHORE_RANGE_CLEAR raw-ISA
    instruction Tile emits at the kernel tail ("ISA wrong length"). Replace it
    with per-semaphore EventSemaphore sem-wr-imm 0 writes (same semantics)."""
    import json

    j = json.loads(bir)

    MAX_WAITS = 1

    def patch_list(insts):
        out = []
        for i in insts:
            si = i.get("sync_info") if isinstance(i, dict) else None
            if si and len(si.get("on_wait") or []) > MAX_WAITS:
                waits = si["on_wait"]
                for k, wt in enumerate(waits[: len(waits) - MAX_WAITS]):
                    out.append(
                        {
                            "debug": i.get("debug", 0),
                            "engine": i["engine"],
                            "ins": [],
                            "outs": [],
                            "name": f"{i['name']}_prewait_{k}",
                            "opcode": "EventSemaphore",
                            "sync_info": {"on_wait": [wt], "on_update": []},
                        }
                    )
                si["on_wait"] = waits[len(waits) - MAX_WAITS :]
            if (
                isinstance(i, dict)
                and i.get("opcode") == "ISA"
                and i.get("op_name") == "EVENT_SEMAPHORE_RANGE_CLEAR"
            ):
                ad = i["ant_dict"]
                first, last = ad["range_first"], ad["range_last"]
                for s in range(first, last + 1):
                    out.append(
                        {
                            "debug": i.get("debug", 0),
                            "engine": i["engine"],
                            "ins": [],
                            "outs": [],
                            "name": f"{i['name']}_semclr_{s}",
                            "opcode": "EventSemaphore",
                            "sync_info": {
                                "on_wait": [],
                                "on_update": [
                                    {
                                        "ant_name": f"semclr_{s}",
                                        "id": s,
                                        "sync_type": "semaphore",
                                        "update_mode": "sem-wr-imm",
                                        "update_value": 0,
                                    }
                                ],
                            },
                        }
                    )
            else:
                out.append(i)
        return out

    def walk(o):
        if isinstance(o, dict):
            if "instructions" in o:
                o["instructions"] = patch_list(o["instructions"])
            for v in o.values():
                walk(v)
        elif isinstance(o, list):
            for v in o:
                walk(v)

    walk(j)
    return json.dumps(j).encode()


def _enable_bir_patch(nc):
    orig = nc.to_json_bytes
    nc.to_json_bytes = lambda: _patch_bir_sem_clear(orig())


class Cfg:
    def __init__(self, nodes_per_core, n_ctx, Cb, G=5, NB=3, fold=True):
        self.nodes_per_core = nodes_per_core
        self.n_ctx = n_ctx
        self.nblk = math.ceil(nodes_per_core / P)
        self.npad = self.nblk * P
        self.Cb = list(Cb)  # chunks per block (shared across cores)
        assert len(self.Cb) == self.nblk
        self.chunk_base = np.concatenate([[0], np.cumsum(self.Cb)]).astype(np.int64)
        self.S_total = int(self.chunk_base[-1])
        self.G = G
        self.NB = NB
        self.fold = fold
        # groups: list of (block_lo, block_hi)
        self.groups = [
            (g, min(g + G, self.nblk)) for g in range(0, self.nblk, G)
        ]
        self.S_max = max(
            int(self.chunk_base[bh] - self.chunk_base[bl]) for bl, bh in self.groups
        )


# ---------------------------------------------------------------- host prep --


def _wrap16(vals):
    """Pack an int16 index vector into the [128, ceil(n/16)] wrapped layout
    (idx i at [i%16, i//16], replicated over the 8 groups of 16 partitions)."""
    n = len(vals)
    cols = (n + 15) // 16
    pad = np.zeros(cols * 16, np.int16)
    pad[:n] = vals
    w = pad.reshape(cols, 16).T  # [16, cols]
    return np.tile(w, (8, 1))  # [128, cols]


def prep(inputs, n_cores=N_CORES, G=5, NB=3):
    hi = np.asarray(inputs["hi"]).astype(np.int64)
    wi = np.asarray(inputs["wi"]).astype(np.int64)
    agts = np.asarray(inputs["agts"], np.float32)
    ctx = np.asarray(inputs["ctx"], np.float32)
    agt_ctrs = np.asarray(inputs["agt_ctrs"], np.float32)
    ctx_ctrs = np.asarray(inputs["ctx_ctrs"], np.float32)

    n_agt = agts.shape[0]
    n_ctx = ctx.shape[0]
    npc = n_agt // n_cores
    assert npc * n_cores == n_agt
    nblk = math.ceil(npc / P)
    npad = nblk * P

    fold = (
        all(np.allclose(inputs[k], 1.0) for k in ("g_dist", "g_q", "g_c1", "g_n", "g_lin"))
        and all(
            np.allclose(inputs[k], 0.0) for k in ("b_dist", "b_q", "b_c1", "b_n", "b_lin")
        )
    )

    core_of = hi // npc
    # per-core sorted edge lists and per-block counts
    per_core = []
    cnt = np.zeros((n_cores, nblk), np.int64)
    for m in range(n_cores):
        eids = np.nonzero(core_of == m)[0]
        hl = hi[eids] - m * npc
        order = np.argsort(hl, kind="stable")
        eids = eids[order]
        hl = hl[order]
        blk = hl // P
        c = np.bincount(blk, minlength=nblk)
        cnt[m] = c
        per_core.append((eids, hl, blk))

    Cb = np.maximum(1, np.ceil(cnt.max(axis=0) / P).astype(np.int64))
    cfg = Cfg(npc, n_ctx, Cb, G=G, NB=NB, fold=fold)
    S = cfg.S_total
    NS = S * P

    # block/group id per slot
    block_of_chunk = np.repeat(np.arange(nblk), Cb)
    group_of_block = np.zeros(nblk, np.int64)
    for gi, (bl, bh) in enumerate(cfg.groups):
        group_of_block[bl:bh] = gi
    gfirst = np.array([bl for bl, _ in cfg.groups])  # first block of group
    block_of_slot = np.repeat(block_of_chunk, P)
    gbase_of_slot = gfirst[group_of_block[block_of_slot]] * P  # node base of group

    ctx_bf16 = ctx.astype(NPBF16)

    w = {}
    w["Wd1"] = np.asarray(inputs["W_dist1"], np.float32).astype(NPBF16)  # [2,128]
    w["b1"] = np.asarray(inputs["b_dist1"], np.float32).reshape(P, 1)
    w["Wd2"] = np.asarray(inputs["W_dist2"], np.float32).astype(NPBF16)
    w["Wq"] = np.asarray(inputs["W_q"], np.float32).astype(NPBF16)
    wc1 = np.asarray(inputs["W_c1"], np.float32)
    w["Wc1a"] = wc1[0:P].astype(NPBF16)
    w["Wc1b"] = wc1[P : 2 * P].astype(NPBF16)
    w["Wc1c"] = wc1[2 * P : 3 * P].astype(NPBF16)
    w["Wc2"] = np.asarray(inputs["W_c2"], np.float32).astype(NPBF16)
    w["Wagt"] = np.asarray(inputs["W_agt"], np.float32).astype(NPBF16)
    w["Wlin"] = np.asarray(inputs["W_lin"], np.float32).astype(NPBF16)
    w["ident"] = np.eye(P, dtype=np.float32)
    if not fold:
        for nm, key in [
            ("g_dist_t", "g_dist"), ("b_dist_t", "b_dist"),
            ("g_q_t", "g_q"), ("b_q_t", "b_q"),
            ("g_c1_t", "g_c1"), ("b_c1_t", "b_c1"),
            ("g_n_t", "g_n"), ("b_n_t", "b_n"),
            ("g_lin_t", "g_lin"), ("b_lin_t", "b_lin"),
        ]:
            w[nm] = np.tile(np.asarray(inputs[key], np.float32).reshape(1, P), (P, 1))

    # static iota16 for sbuf-source transpose gathers
    iota = _wrap16(np.arange(cfg.S_max * P, dtype=np.int16))

    in_maps = []
    for m in range(n_cores):
        eids, hl, blk = per_core[m]
        c = cnt[m]
        first_slot = (cfg.chunk_base[:-1] * P)[blk]  # slot base of edge's block
        within = np.arange(len(eids)) - np.repeat(
            np.concatenate([[0], np.cumsum(c)])[:-1], c
        )
        slot = first_slot + within

        d0 = agt_ctrs[hi[eids]] - ctx_ctrs[wi[eids]]  # [ne, 2]
        d0T = np.zeros((2, NS), np.float32)
        d0T[:, slot] = d0.T
        d0T = d0T.astype(NPBF16)

        wi_flat = np.zeros(NS, np.int64)
        wi_flat[slot] = wi[eids]
        # pre-gathered ctx rows in slot-interleaved layout [p, chunk, d]
        ctx_slab = np.ascontiguousarray(
            ctx_bf16[wi_flat.reshape(S, P)].transpose(1, 0, 2)
        )

        hrel = hl - P * blk  # in [0,128)
        oh = np.zeros((P, NS), NPBF16)
        # rhs tile for the scatter matmul: [edge-in-chunk (partition), node-in-block]
        oh[slot % P, (slot // P) * P + hrel] = NPBF16(1.0)

        qi_flat = np.zeros(NS, np.int64)
        qi_flat[slot] = hl - gbase_of_slot[slot]
        qi_s = _wrap16(qi_flat.astype(np.int16))  # [128, NS/16]

        agts_pad = np.zeros((npad, P), np.float32)
        agts_pad[:npc] = agts[m * npc : (m + 1) * npc]

        im = dict(
            d0T=d0T,
            oh=oh,
            ctx_slab=ctx_slab,
            qi_s=qi_s,
            iota16=iota,
            agtsT=np.ascontiguousarray(agts_pad.T).astype(NPBF16),
            agts_res=agts_pad,
        )
        im.update(w)
        in_maps.append(im)
    return cfg, in_maps


# ------------------------------------------------------------ graph builder --


def _gn_stats(nc, pools, src_ap):
    """bn stats over free dim of src_ap [128, 128] -> (rs, neg_mu_rs) [128,1]."""
    small = pools["small"]
    stats = small.tile([P, 6], F32, tag="stats")
    nc.vector.bn_stats(stats[:], src_ap)
    mv = small.tile([P, 2], F32, tag="mv")
    nc.vector.bn_aggr(mv[:], stats[:])
    rs = small.tile([P, 1], F32, tag="rs")
    nc.scalar.activation(
        rs[:], mv[:, 1:2], mybir.ActivationFunctionType.Sqrt,
        bias=pools["eps"][:], scale=1.0,
    )
    nc.vector.reciprocal(rs[:], rs[:])
    nmr = small.tile([P, 1], F32, tag="nmr")
    nc.vector.tensor_scalar(
        out=nmr[:], in0=mv[:, 0:1], scalar1=rs[:], scalar2=-1.0,
        op0=mybir.AluOpType.mult, op1=mybir.AluOpType.mult,
    )
    return rs, nmr


def _gn_apply(nc, pools, out_ap, src_ap, rs, nmr, relu, gt=None, bt=None):
    """out = [relu](gn(src)) with optional per-channel g/b tiles."""
    if gt is None:
        func = (
            mybir.ActivationFunctionType.Relu
            if relu
            else mybir.ActivationFunctionType.Identity
        )
        nc.scalar.activation(out_ap, src_ap, func, bias=nmr[:], scale=rs[:])
    else:
        sb = pools["sb"]
        xn = sb.tile([P, P], F32, tag="xn")
        nc.scalar.activation(
            xn[:], src_ap, mybir.ActivationFunctionType.Identity,
            bias=nmr[:], scale=rs[:],
        )
        x2 = sb.tile([P, P], F32, tag="xn2")
        nc.vector.tensor_tensor(out=x2[:], in0=xn[:], in1=gt[:], op=mybir.AluOpType.mult)
        if relu:
            nc.vector.tensor_tensor(out=xn[:], in0=x2[:], in1=bt[:], op=mybir.AluOpType.add)
            nc.vector.tensor_scalar(
                out=out_ap, in0=xn[:], scalar1=0.0, scalar2=None,
                op0=mybir.AluOpType.max,
            )
        else:
            nc.vector.tensor_tensor(out=out_ap, in0=x2[:], in1=bt[:], op=mybir.AluOpType.add)


def build(cfg: Cfg):
    import os

    stage = int(os.environ.get("KSTAGE", "5"))
    single_packet = os.environ.get("KSP", "0") == "1"
    nc = bass.Bass()
    npad, nblk, S = cfg.npad, cfg.nblk, cfg.S_total
    NS = S * P

    d0T_d = nc.declare_dram_parameter("d0T", [2, NS], BF16, isOutput=False)
    oh_d = nc.declare_dram_parameter("oh", [P, NS], BF16, isOutput=False)
    ctxs_d = nc.declare_dram_parameter("ctx_slab", [P, S, P], BF16, isOutput=False)
    qi_d = nc.declare_dram_parameter("qi_s", [P, NS // 16], I16, isOutput=False)
    iota_d = nc.declare_dram_parameter("iota16", [P, cfg.S_max * 8], I16, isOutput=False)
    agtsT_d = nc.declare_dram_parameter("agtsT", [P, npad], BF16, isOutput=False)
    res_d = nc.declare_dram_parameter("agts_res", [npad, P], F32, isOutput=False)
    wd = {}
    wd["Wd1"] = nc.declare_dram_parameter("Wd1", [2, P], BF16, isOutput=False)
    wd["b1"] = nc.declare_dram_parameter("b1", [P, 1], F32, isOutput=False)
    for nm in ["Wd2", "Wq", "Wc1a", "Wc1b", "Wc1c", "Wc2", "Wagt", "Wlin"]:
        wd[nm] = nc.declare_dram_parameter(nm, [P, P], BF16, isOutput=False)
    wd["ident"] = nc.declare_dram_parameter("ident", [P, P], F32, isOutput=False)
    gb_names = []
    if not cfg.fold:
        gb_names = [
            "g_dist_t", "b_dist_t", "g_q_t", "b_q_t", "g_c1_t", "b_c1_t",
            "g_n_t", "b_n_t", "g_lin_t", "b_lin_t",
        ]
        for nm in gb_names:
            wd[nm] = nc.declare_dram_parameter(nm, [P, P], F32, isOutput=False)
    out_d = nc.declare_dram_parameter("out", [npad, P], F32, isOutput=True)

    with tile.TileContext(nc) as tc:
        import contextlib

        with contextlib.ExitStack() as ctx:
            const = ctx.enter_context(tc.tile_pool(name="const", bufs=1))
            qn_pool = ctx.enter_context(tc.tile_pool(name="qn", bufs=2))
            slab = ctx.enter_context(tc.tile_pool(name="slab", bufs=2))
            small = ctx.enter_context(tc.tile_pool(name="small", bufs=12))
            sb = ctx.enter_context(tc.tile_pool(name="sb", bufs=4))
            nsb = ctx.enter_context(tc.tile_pool(name="nsb", bufs=2))
            ps_edge = ctx.enter_context(tc.tile_pool(name="ps_e", bufs=4, space="PSUM"))
            ps_acc = ctx.enter_context(tc.tile_pool(name="ps_a", bufs=2, space="PSUM"))
            ps_node = ctx.enter_context(tc.tile_pool(name="ps_n", bufs=2, space="PSUM"))
            pools = {"small": small, "sb": sb}

            eps_t = const.tile([P, 1], F32, tag="eps")
            nc.vector.memset(eps_t[:], EPS)
            pools["eps"] = eps_t

            nc.gpsimd.load_library(library_config.mlp)

            # one Pool register per distinct gather size (to_reg per call
            # exhausts the register file)
            nidx_regs = {}

            def nidx_reg(n):
                if n not in nidx_regs:
                    nidx_regs[n] = nc.gpsimd.to_reg(n)
                return nidx_regs[n]

            # ---- constants
            wt = {}
            for nm, d in wd.items():
                shape = list(d.shape)
                t = const.tile(shape, d.dtype, tag=f"w_{nm}")
                nc.sync.dma_start(out=t[:], in_=d[:, :])
                wt[nm] = t
            iota_t = const.tile([P, cfg.S_max * 8], I16, tag="iota")
            nc.sync.dma_start(out=iota_t[:], in_=iota_d[:, :])

            def GT(name):
                return wt[name] if not cfg.fold else None

            # ---- per group
            for gi, (bl, bh) in enumerate(cfg.groups):
                gnb = bh - bl  # blocks in group
                k0 = int(cfg.chunk_base[bl])
                k1 = int(cfg.chunk_base[bh])
                Sg = k1 - k0
                NSg = Sg * P

                # agts^T for this group's blocks
                agtsT_g = qn_pool.tile([P, cfg.G * P], BF16, tag="agtsT_g")
                nc.sync.dma_start(
                    out=agtsT_g[:, : gnb * P], in_=agtsT_d[:, bl * P : bh * P]
                )

                # ---- qn precompute for the group's node blocks
                qn_t = qn_pool.tile([P, cfg.G * P], BF16, tag="qn_t")
                for j in range(gnb):
                    qpre = ps_node.tile([P, cfg.NB * P], F32, tag="node_ps")
                    nc.tensor.matmul(
                        qpre[:, :P],
                        agtsT_g[:, j * P : (j + 1) * P],
                        wt["Wq"][:],
                        start=True,
                        stop=True,
                    )
                    rs, nmr = _gn_stats(nc, pools, qpre[:, :P])
                    _gn_apply(
                        nc, pools, qn_t[:, j * P : (j + 1) * P], qpre[:, :P],
                        rs, nmr, relu=True, gt=GT("g_q_t"), bt=GT("b_q_t"),
                    )

                # ---- per-group slabs
                d0T_t = slab.tile([2, cfg.S_max * P], BF16, tag="d0T")
                nc.sync.dma_start(out=d0T_t[:, :NSg], in_=d0T_d[:, k0 * P : k1 * P])
                oh_t = slab.tile([P, cfg.S_max * P], BF16, tag="oh")
                nc.sync.dma_start(out=oh_t[:, :NSg], in_=oh_d[:, k0 * P : k1 * P])
                qi_t = slab.tile([P, cfg.S_max * 8], I16, tag="qi")
                nc.sync.dma_start(out=qi_t[:, : Sg * 8], in_=qi_d[:, k0 * 8 : k1 * 8])

                if stage < 2:
                    # debug: dump qn tile and stop this group
                    nc.gpsimd.dma_start(
                        out=out_d[bl * P : bh * P, :].rearrange(
                            "(j p) d -> p j d", p=P
                        ),
                        in_=qn_t[:, : gnb * P].rearrange("p (j d) -> p j d", d=P),
                    )
                    continue

                # ---- gathers
                ctx_rows = slab.tile([P, cfg.S_max, P], BF16, tag="ctx_rows")
                nc.sync.dma_start(out=ctx_rows[:, :Sg, :], in_=ctxs_d[:, k0:k1, :])
                ctxT = slab.tile([P, 1, cfg.S_max * P], BF16, tag="ctxT")
                nc.gpsimd.dma_gather(
                    out_ap=ctxT[:, :, :NSg],
                    in_ap=ctx_rows[:, :Sg, :],
                    idxs_ap=iota_t[:, : Sg * 8],
                    num_idxs=NSg,
                    num_idxs_reg=nidx_reg(NSg),
                    elem_size=P,
                    transpose=True,
                    sbuf_tokens_per_rank=P,
                    sbuf_free_dim_per_rank=P * 2,
                    single_packet=single_packet,
                )
                qnT = slab.tile([P, 1, cfg.S_max * P], BF16, tag="qnT")
                nc.gpsimd.dma_gather(
                    out_ap=qnT[:, :, :NSg],
                    in_ap=qn_t[:, : gnb * P],
                    idxs_ap=qi_t[:, : Sg * 8],
                    num_idxs=NSg,
                    num_idxs_reg=nidx_reg(NSg),
                    elem_size=P,
                    transpose=True,
                    sbuf_tokens_per_rank=P,
                    sbuf_free_dim_per_rank=P * 2,
                    single_packet=single_packet,
                )

                if stage < 3:
                    nc.gpsimd.dma_start(
                        out=out_d[bl * P : bh * P, :].rearrange(
                            "(j p) d -> p j d", p=P
                        ),
                        in_=qnT[:, 0, : gnb * P].rearrange(
                            "p (j d) -> p j d", d=P
                        ),
                    )
                    continue

                # ---- pass 1: dist MLP -> dfeat
                dfeat = slab.tile([P, cfg.S_max, P], BF16, tag="dfeat")
                for k in range(Sg):
                    ksl = slice(k * P, (k + 1) * P)
                    h1T_ps = ps_edge.tile([P, P], F32, tag="eps")
                    nc.tensor.matmul(
                        h1T_ps[:], wt["Wd1"][:], d0T_t[:, ksl], start=True, stop=True
                    )
                    h1T = sb.tile([P, P], BF16, tag="h1T")
                    nc.scalar.activation(
                        h1T[:], h1T_ps[:], mybir.ActivationFunctionType.Relu,
                        bias=wt["b1"][:], scale=1.0,
                    )
                    h2_ps = ps_edge.tile([P, P], F32, tag="eps")
                    nc.tensor.matmul(h2_ps[:], h1T[:], wt["Wd2"][:], start=True, stop=True)
                    rs, nmr = _gn_stats(nc, pools, h2_ps[:])
                    _gn_apply(
                        nc, pools, dfeat[:, k, :], h2_ps[:], rs, nmr, relu=True,
                        gt=GT("g_dist_t"), bt=GT("b_dist_t"),
                    )
                dfeatT = slab.tile([P, 1, cfg.S_max * P], BF16, tag="dfeatT")
                nc.gpsimd.dma_gather(
                    out_ap=dfeatT[:, :, :NSg],
                    in_ap=dfeat[:, :Sg, :],
                    idxs_ap=iota_t[:, : Sg * 8],
                    num_idxs=NSg,
                    num_idxs_reg=nidx_reg(NSg),
                    elem_size=P,
                    transpose=True,
                    sbuf_tokens_per_rank=P,
                    sbuf_free_dim_per_rank=P * 2,
                    single_packet=single_packet,
                )

                if stage < 4:
                    nc.gpsimd.dma_start(
                        out=out_d[bl * P : bh * P, :].rearrange(
                            "(j p) d -> p j d", p=P
                        ),
                        in_=dfeatT[:, 0, : gnb * P].rearrange(
                            "p (j d) -> p j d", d=P
                        ),
                    )
                    continue

                # ---- pass 2 + scatter + node epilogue, per node-batch
                nbatches = [
                    (j0, min(j0 + cfg.NB, gnb)) for j0 in range(0, gnb, cfg.NB)
                ]
                for (j0, j1) in nbatches:
                    nbw = j1 - j0
                    accT = ps_acc.tile([P, cfg.NB * P], F32, tag="accT")
                    for j in range(j0, j1):
                        b = bl + j
                        cb0 = int(cfg.chunk_base[b]) - k0
                        cbn = int(cfg.Cb[b])
                        asl = slice((j - j0) * P, (j - j0 + 1) * P)
                        for ci in range(cbn):
                            k = cb0 + ci
                            ksl = slice(k * P, (k + 1) * P)
                            c1_ps = ps_edge.tile([P, P], F32, tag="eps")
                            nc.tensor.matmul(
                                c1_ps[:], dfeatT[:, 0, ksl], wt["Wc1a"][:],
                                start=True, stop=False,
                            )
                            nc.tensor.matmul(
                                c1_ps[:], qnT[:, 0, ksl], wt["Wc1b"][:],
                                start=False, stop=False,
                            )
                            nc.tensor.matmul(
                                c1_ps[:], ctxT[:, 0, ksl], wt["Wc1c"][:],
                                start=False, stop=True,
                            )
                            rs, nmr = _gn_stats(nc, pools, c1_ps[:])
                            c1sb = sb.tile([P, P], BF16, tag="c1sb")
                            _gn_apply(
                                nc, pools, c1sb[:], c1_ps[:], rs, nmr, relu=True,
                                gt=GT("g_c1_t"), bt=GT("b_c1_t"),
                            )
                            nc.tensor.matmul(
                                accT[:, asl], c1sb[:], oh_t[:, ksl],
                                start=(ci == 0), stop=(ci == cbn - 1),
                            )

                    # node epilogue for blocks [bl+j0, bl+j1)
                    accT_sb = nsb.tile([P, cfg.NB * P], BF16, tag="accT_sb")
                    nc.vector.tensor_copy(
                        accT_sb[:, : nbw * P], accT[:, : nbw * P]
                    )
                    if stage < 5:
                        nc.gpsimd.dma_start(
                            out=out_d[
                                (bl + j0) * P : (bl + j1) * P, :
                            ].rearrange("(j p) d -> p j d", p=P),
                            in_=accT_sb[:, : nbw * P].rearrange(
                                "p (j d) -> p j d", d=P
                            ),
                        )
                        continue
                    a_ps = ps_node.tile([P, cfg.NB * P], F32, tag="node_ps")
                    for j in range(j0, j1):
                        asl = slice((j - j0) * P, (j - j0 + 1) * P)
                        jsl = slice(j * P, (j + 1) * P)
                        nc.tensor.matmul(
                            a_ps[:, asl], accT_sb[:, asl], wt["Wc2"][:],
                            start=True, stop=False,
                        )
                        nc.tensor.matmul(
                            a_ps[:, asl], agtsT_g[:, jsl], wt["Wagt"][:],
                            start=False, stop=True,
                        )
                    a_sb = nsb.tile([P, cfg.NB * P], F32, tag="a_sb")
                    for j in range(j0, j1):
                        asl = slice((j - j0) * P, (j - j0 + 1) * P)
                        rs, nmr = _gn_stats(nc, pools, a_ps[:, asl])
                        _gn_apply(
                            nc, pools, a_sb[:, asl], a_ps[:, asl], rs, nmr,
                            relu=True, gt=GT("g_n_t"), bt=GT("b_n_t"),
                        )
                    y_ps = ps_node.tile([P, cfg.NB * P], F32, tag="node_ps")
                    for j in range(j0, j1):
                        asl = slice((j - j0) * P, (j - j0 + 1) * P)
                        aT_ps = ps_node.tile([P, cfg.NB * P], F32, tag="node_ps")
                        nc.tensor.transpose(
                            aT_ps[:, :P], a_sb[:, asl], wt["ident"][:]
                        )
                        aT_sb = sb.tile([P, P], BF16, tag="aT_sb")
                        nc.vector.tensor_copy(aT_sb[:], aT_ps[:, :P])
                        nc.tensor.matmul(
                            y_ps[:, asl], aT_sb[:], wt["Wlin"][:],
                            start=True, stop=True,
                        )
                    yn = nsb.tile([P, cfg.NB * P], F32, tag="yn")
                    for j in range(j0, j1):
                        asl = slice((j - j0) * P, (j - j0 + 1) * P)
                        rs, nmr = _gn_stats(nc, pools, y_ps[:, asl])
                        _gn_apply(
                            nc, pools, yn[:, asl], y_ps[:, asl], rs, nmr,
                            relu=False, gt=GT("g_lin_t"), bt=GT("b_lin_t"),
                        )
                    res_t = nsb.tile([P, cfg.NB, P], F32, tag="res")
                    r0 = (bl + j0) * P
                    r1 = (bl + j1) * P
                    nc.sync.dma_start(
                        out=res_t[:, :nbw, :],
                        in_=res_d[r0:r1, :].rearrange("(j p) d -> p j d", p=P),
                    )
                    o_t = nsb.tile([P, cfg.NB, P], F32, tag="o_t")
                    nc.vector.tensor_tensor(
                        out=o_t[:, :nbw, :],
                        in0=yn[:, : nbw * P].rearrange("p (j d) -> p j d", d=P),
                        in1=res_t[:, :nbw, :],
                        op=mybir.AluOpType.add,
                    )
                    oo_t = nsb.tile([P, cfg.NB, P], F32, tag="oo_t")
                    nc.scalar.activation(
                        oo_t[:, :nbw, :], o_t[:, :nbw, :],
                        mybir.ActivationFunctionType.Relu,
                    )
                    nc.sync.dma_start(
                        out=out_d[r0:r1, :].rearrange("(j p) d -> p j d", p=P),
                        in_=oo_t[:, :nbw, :],
                    )
    # raw Bass skips Bacc's extended-inst codegen pass; without it the NEFF
    # compiler sees empty .instr bytes for ISA subclasses
    mybir.codegen_inst_isa_subclasses(nc)
    return nc


# ------------------------------------------------------------------- runner --

LAST_RESULTS = None


def kernel(**inputs):
    global LAST_RESULTS
    cfg, in_maps = prep(inputs)
    nc = build(cfg)
    _enable_bir_patch(nc)
    res = run_bass_kernel_spmd(nc, in_maps, core_ids=list(range(N_CORES)))
    LAST_RESULTS = res
    npc = cfg.nodes_per_core
    out = np.concatenate(
        [np.asarray(res.results[m]["out"])[:npc] for m in range(N_CORES)], axis=0
    )
    return out.astype(np.float32)
